# revision 5
# baseline (speedup 1.0000x reference)
"""Trainium2 Bass kernel for nn_EphapticCoupling_51857435132573 (v2).

Math: for x[B,M,D], w[D,K=3] the reference collapses to a rank-3 update
    out[b,m,d] = x[b,m,d] + sum_k U[b,m,k] * wt[k,d]
    U  = decay @ S,  S = [T - x[..,-1], T, T - x[..,0]],  T = x.sum(-1)
    wt = (0.1/D) * w.T   (scale folded into dk here)

I/O precision: gate is rel_err < 2e-2; host feeds x fp16 and reads the
output fp16 (8 MiB in + 8 MiB out per core).

v2 changes vs v1 (measured on HW, see session notes):
  - Inputs split across BOTH HWDGE rings (qSP + qAct): in-dispatches carry
    no waits so they never block ACT's later compute; 2 rings ~halve the
    input stream time (single ring measured 321 GB/s, HBM/NC cap ~358).
  - Outputs all on the SP ring behind its inputs, grouped in pairs
    (3-dim APs), singles for the last two tiles to trim the epilogue.
  - Row sums split ACT (activation+accum, 2.28us) / DVE (tensor_reduce,
    2.29us) per SUM_ACT table to balance engines.
  - corr = U @ wt stays on PE (fp32 PSUM writeout is the HW floor,
    64 x ~615ns). U^T matmuls grouped via quadrant-spread lhsT
    (S_spread[128, 99], tiles at partition bases 0/32/64/96): one matmul
    + ONE [99,128] psum->sbuf cast per group (ramp 1,1,2,4,4,4 keeps
    tile0 latency low).
  - x += corr: per ADD_COPY table either direct DVE tensor_tensor from
    PSUM (1213ns per [128,1024], PSUM caps DVE at 1x) or ACT copies
    PSUM->fp16 scratch (4 x ~570ns) + DVE all-16-bit add (1218ns per
    [128,2048], 2x) — trades ACT idle time for DVE headroom.
  - Pool (gpsimd) does S builds; it has no PSUM port (HW) and
    scalar_tensor_tensor/partition_broadcast are not in its ISA.
  - TensorTensor on ACT is illegal on TRN2 (walrus NCC_IBIR606); matmul
    PSUM output must be fp32 on TRN2 — both verified, hence this shape.
"""

import numpy as np
import ml_dtypes

import concourse.bass as bass
import concourse.tile as tile
from concourse import mybir
from concourse.bass_utils import run_bass_kernel_spmd

B, M, D, K = 512, 32, 2048, 3
COUPLING_STRENGTH = 0.1
SPATIAL_DECAY = 2.0

N_CORES = 8
B_SH = B // N_CORES          # 64 batches per core
ROWS = B_SH * M              # 2048 rows per core
P = 128                      # SBUF partitions
N_TILES = ROWS // P          # 16
F32 = mybir.dt.float32
F16 = mybir.dt.float16
BF16 = mybir.dt.bfloat16
ALU = mybir.AluOpType
ACTF = mybir.ActivationFunctionType

# ---- tunables -----------------------------------------------------------
# U^T group ramp: first tiles get tiny groups so tile0's corr isn't gated
# on later tiles' sums.
# rust AP base_partition() only allows {0, 32, 64} -> max 3 tiles/group
GROUPS = [[0], [1], [2, 3], [4, 5, 6], [7, 8, 9], [10, 11, 12], [13, 14, 15]]
# input-dispatch ring per tile: 'sp' or 'act' (interleaved for early arrival)
IN_RING = ["sp", "act"] * 8
# which tiles' row sums run on ACT (others: DVE tensor_reduce)
SUM_ACT = [True, False, True, False, True, False, True, False,
           True, False, True, False, True, False, True, False]
# which tiles use the ACT-copy + DVE 16-bit add scheme (others: direct DVE)
ADD_COPY = [False, True, False, True, False, True, False, False,
            False, True, False, True, False, True, False, False]
# output grouping: list of tile-lists; each becomes one SP-ring DMA
OUT_GROUPS = [[0, 1], [2, 3], [4, 5], [6, 7], [8, 9], [10, 11], [12, 13], [14], [15]]

_ENGINE_SEM = {
    "Activation": "Activation",
    "DVE": "DVE",
    "Pool": "Pool",
    "PE": "PE",
}


def _group_of(t):
    for gi, g in enumerate(GROUPS):
        if t in g:
            return gi, g.index(t)
    raise ValueError(t)


def _emit(tc, nc, x, wtq, dk, out):
    with (
        tc.tile_pool(name="const", bufs=1) as const_pool,
        tc.tile_pool(name="xbuf", bufs=1) as xbuf_pool,
        tc.tile_pool(name="small", bufs=1) as small_pool,
        tc.tile_pool(name="psc", bufs=1, space="PSUM") as psc_pool,
    ):
        # Constants ride Pool's software queue: idle at the head.
        dk_sb = const_pool.tile([P, P], BF16)
        nc.gpsimd.dma_start(out=dk_sb, in_=dk)
        wtq_sb = const_pool.tile([67, D], BF16)
        nc.gpsimd.dma_start(out=wtq_sb, in_=wtq)

        x_all = xbuf_pool.tile([P, N_TILES * D], F16)
        t_all = small_pool.tile([P, 2 * N_TILES], F32)
        trash = small_pool.tile([P, D], F16)
        scratch = [
            small_pool.tile([P, D], F16, name=f"scr{i}") for i in range(2)
        ]
        # per-group quadrant-spread S (tile j of group at partition-col 32j)
        s_sp = [
            small_pool.tile([P, 32 * (len(g) - 1) + K], BF16, name=f"ssp{gi}")
            for gi, g in enumerate(GROUPS)
        ]
        u_sb = [
            small_pool.tile([32 * (len(g) - 1) + K, P], BF16, name=f"usb{gi}")
            for gi, g in enumerate(GROUPS)
        ]
        # Four fixed [128,1024] fp32 corr PSUM slots (all 8 banks).
        pscs = [
            psc_pool.tile([P, 1024], F32, name=f"pscs{i}") for i in range(4)
        ]

        # ---- input DMAs: no waits, dispatched first on both rings ----
        n_chunks = [2] + [1] * (N_TILES - 1)
        for t in range(N_TILES):
            eng = nc.sync if IN_RING[t] == "sp" else nc.scalar
            ch = D // n_chunks[t]
            for h in range(n_chunks[t]):
                eng.dma_start(
                    out=x_all[:, t * D + h * ch : t * D + (h + 1) * ch],
                    in_=x[t * P : (t + 1) * P, h * ch : (h + 1) * ch],
                )

        def emit_front(t):
            """Row sum + S build (into the group spread tile) for tile t."""
            x_sl = x_all[:, t * D : (t + 1) * D]
            ch = D // n_chunks[t]
            if SUM_ACT[t]:
                for h in range(n_chunks[t]):
                    nc.scalar.activation(
                        out=trash[:, :ch],
                        in_=x_all[:, t * D + h * ch : t * D + (h + 1) * ch],
                        func=ACTF.Copy,
                        accum_out=t_all[:, 2 * t + h : 2 * t + h + 1],
                    )
            else:
                for h in range(n_chunks[t]):
                    nc.vector.tensor_reduce(
                        out=t_all[:, 2 * t + h : 2 * t + h + 1],
                        in_=x_all[:, t * D + h * ch : t * D + (h + 1) * ch],
                        axis=mybir.AxisListType.X,
                        op=ALU.add,
                    )
            gi, j = _group_of(t)
            st = s_sp[gi][:, 32 * j : 32 * j + K]
            ta = t_all[:, 2 * t : 2 * t + 1]
            if n_chunks[t] == 1:
                nc.gpsimd.tensor_copy(st[:, 1:2], ta)
            else:
                nc.gpsimd.tensor_add(
                    st[:, 1:2], ta, t_all[:, 2 * t + 1 : 2 * t + 2]
                )
            nc.gpsimd.tensor_sub(st[:, 0:1], st[:, 1:2], x_sl[:, D - 1 : D])
            nc.gpsimd.tensor_sub(st[:, 2:3], st[:, 1:2], x_sl[:, 0:1])

        def emit_group_u(gi):
            """U^T for a whole group: one matmul + one cast."""
            g = GROUPS[gi]
            rows = 32 * (len(g) - 1) + K
            ut_ps = pscs[(2 * g[0]) % 4][0:rows, 0:P]
            nc.tensor.matmul(
                ut_ps, lhsT=s_sp[gi][:], rhs=dk_sb, start=True, stop=True
            )
            nc.vector.tensor_copy(u_sb[gi][:], ut_ps)

        def emit_add(t, scr_i):
            """x += corr for tile t (direct DVE or ACT-copy scheme)."""
            x_sl = x_all[:, t * D : (t + 1) * D]
            if ADD_COPY[t]:
                scr = scratch[scr_i]
                for c in range(2):
                    ps = pscs[(2 * t + c) % 4]
                    for n in range(2):
                        nc.scalar.activation(
                            out=scr[:, c * 1024 + n * 512 : c * 1024 + (n + 1) * 512],
                            in_=ps[:, n * 512 : (n + 1) * 512],
                            func=ACTF.Copy,
                        )
                nc.vector.tensor_tensor(
                    out=x_sl, in0=x_sl, in1=scr[:], op=ALU.add
                )
            else:
                for c in range(2):
                    ps = pscs[(2 * t + c) % 4]
                    nc.vector.tensor_add(
                        x_sl[:, c * 1024 : (c + 1) * 1024],
                        x_sl[:, c * 1024 : (c + 1) * 1024],
                        ps,
                    )

        def emit_corr(t):
            gi, j = _group_of(t)
            u_sl = u_sb[gi][32 * j : 32 * j + K, :]
            wt_sl = wtq_sb[32 * j : 32 * j + K, :]
            for c in range(2):
                ps = pscs[(2 * t + c) % 4]
                for n in range(2):
                    col0 = c * 1024 + n * 512
                    nc.tensor.matmul(
                        ps[:, n * 512 : (n + 1) * 512],
                        lhsT=u_sl,
                        rhs=wt_sl[:, col0 : col0 + 512],
                        start=True,
                        stop=True,
                    )

        # ---- software pipeline ----
        # front runs 2 tiles ahead; each group's U^T emitted right after
        # its last member's front.
        emitted_u = set()

        def maybe_emit_u(upto_tile):
            for gi, g in enumerate(GROUPS):
                if gi not in emitted_u and g[-1] <= upto_tile:
                    emit_group_u(gi)
                    emitted_u.add(gi)

        emit_front(0)
        emit_front(1)
        maybe_emit_u(1)
        scr_i = 0
        for t in range(N_TILES):
            if t + 2 < N_TILES:
                emit_front(t + 2)
                maybe_emit_u(t + 2)
            emit_corr(t)
            emit_add(t, scr_i)
            if ADD_COPY[t]:
                scr_i ^= 1
        # ---- outputs: all on the SP ring, grouped ----
        # Multi-tile groups need an explicit (p, tile, d) DRAM AP: the
        # natural out[rows, :] slice iterates tile-major while the SBUF
        # side iterates partition-major.
        for grp in OUT_GROUPS:
            t0 = grp[0]
            nt = len(grp)
            if nt == 1:
                dst = out.tensor[t0 * P : (t0 + 1) * P, :]
            else:
                dst = bass.AP(
                    out.tensor,
                    t0 * P * D,
                    [[D, P], [P * D, nt], [1, D]],
                )
            nc.sync.dma_start(
                out=dst,
                in_=x_all[:, t0 * D : (t0 + nt) * D],
            )


_NC_CACHE = None


def _build_nc():
    global _NC_CACHE
    if _NC_CACHE is not None:
        return _NC_CACHE
    nc = bass.Bass()
    x = nc.declare_dram_parameter("x", [ROWS, D], F16, isOutput=False)
    wtq = nc.declare_dram_parameter("wtq", [67, D], BF16, isOutput=False)
    dk = nc.declare_dram_parameter("dk", [P, P], BF16, isOutput=False)
    out = nc.declare_dram_parameter("out", [ROWS, D], F16, isOutput=True)
    with tile.TileContext(nc) as tc:
        _emit(tc, nc, x[:], wtq[:], dk[:], out[:])
    _sanitize_waits(nc)
    _NC_CACHE = nc
    return nc


def _sanitize_waits(nc):
    """One semaphore wait per instruction (HW has a single wait slot).

    1. Drop PE-sem self-waits from matmuls (PE executes in order; PSUM
       writes serialize through PE's single write port).
    2. Drop engine self-waits (engine FIFO order implies them).
    3. Split any remaining multi-wait instruction into standalone
       InstEventSemaphore ops on the same queue.
    """
    from concourse import mybir as _mb

    skip = ("InstEventSemaphore", "InstAllEngineBarrier")
    for f in nc.m.functions:
        for bb in f.blocks:
            idx = 0
            insts = bb.instructions
            while idx < len(insts):
                inst = insts[idx]
                si = inst.sync_info
                cls = type(inst).__name__
                eng = str(inst.engine)
                if si is not None and si.on_wait:
                    waits = list(si.on_wait)
                    own = _ENGINE_SEM.get(eng.split(".")[-1])
                    if own is not None and cls not in skip:
                        waits = [
                            w for w in waits if not w.ant_name.startswith(own)
                        ] or waits[-1:]
                    if len(waits) != len(si.on_wait):
                        si = _mb.SyncInfo(
                            on_wait=waits, on_update=list(si.on_update)
                        )
                        inst.sync_info = si
                if (
                    si is None
                    or not si.on_wait
                    or len(si.on_wait) < 2
                    or cls in skip
                ):
                    idx += 1
                    continue
                waits = list(si.on_wait)
                if cls == "InstMatmult":
                    kept = [w for w in waits if not w.ant_name.startswith("PE")]
                    if kept:
                        waits = kept
                n_new = 0
                for w in waits[:-1]:
                    ev = _mb.InstEventSemaphore(
                        name=nc.get_next_instruction_name(), ins=[], outs=[]
                    )
                    ev.engine = inst.engine
                    ev.sync_info = _mb.SyncInfo(on_wait=[w], on_update=[])
                    nc.register_instruction(ev)
                    insts.insert(idx + n_new, ev)
                    n_new += 1
                inst.sync_info = _mb.SyncInfo(
                    on_wait=[waits[-1]], on_update=list(si.on_update)
                )
                idx += n_new + 1


def _host_constants(w: np.ndarray):
    # wt rows replicated at partition bases 0/32/64/96 (quadrant rhs for
    # the per-tile corr matmuls); scale 0.1/D folded into dk instead so
    # wt stays in a good bf16 range.
    wt = w.T.astype(np.float32)  # [3, D]
    wtq = np.zeros((67, D), dtype=np.float32)
    for j in range(3):
        wtq[32 * j : 32 * j + K, :] = wt
    idx = np.arange(M)
    dec = np.exp(-np.abs(idx[:, None] - idx[None, :]) / SPATIAL_DECAY)
    dec = (dec * (1.0 - np.eye(M))) * (COUPLING_STRENGTH / D)
    dk = np.kron(np.eye(P // M, dtype=np.float64), dec).astype(np.float32)
    return (
        np.ascontiguousarray(wtq.astype(ml_dtypes.bfloat16)),
        np.ascontiguousarray(dk.astype(ml_dtypes.bfloat16)),
    )


def _in_maps(x: np.ndarray, w: np.ndarray):
    wtq, dk = _host_constants(np.asarray(w, dtype=np.float32))
    x16 = np.asarray(x, dtype=np.float16).reshape(N_CORES, ROWS, D)
    return [
        {"x": np.ascontiguousarray(x16[i]), "wtq": wtq, "dk": dk}
        for i in range(N_CORES)
    ]


def kernel(x: np.ndarray, w: np.ndarray, _results_out: list | None = None) -> np.ndarray:
    nc = _build_nc()
    res = run_bass_kernel_spmd(nc, _in_maps(x, w), core_ids=list(range(N_CORES)))
    if _results_out is not None:
        _results_out.append(res)
    out = np.concatenate(
        [np.asarray(res.results[i]["out"]).reshape(B_SH, M, D) for i in range(N_CORES)],
        axis=0,
    )
    return out.astype(np.float32)


# revision 8
# speedup vs baseline: 1.1808x; 1.1808x over previous
"""Trainium2 Bass kernel for nn_EphapticCoupling_51857435132573 (v2).

Math: for x[B,M,D], w[D,K=3] the reference collapses to a rank-3 update
    out[b,m,d] = x[b,m,d] + sum_k U[b,m,k] * wt[k,d]
    U  = decay @ S,  S = [T - x[..,-1], T, T - x[..,0]],  T = x.sum(-1)
    (0.1/D scale folded into dk)

I/O precision: gate is rel_err < 2e-2; host feeds x fp16 and reads the
output fp16 (8 MiB in + 8 MiB out per core).

Architecture (every cost HW-measured this session):
  - Inputs split across BOTH HWDGE rings (12 tiles qSP + 4 tiles qAct;
    single ring sustains ~321 GB/s vs ~358 HBM/NC cap). In-dispatches
    carry no waits so ACT's 4 never delay its compute by much.
  - dk + wt consts ride the SP ring right after tile0 (44 KiB; the
    gpsimd software queue measured only ~10 B/ns and starved corr).
    wt is [3,2048]; Pool replicates it to partition bases 32/64 via
    SBUF->SBUF DMAs for the quadrant corr matmuls.
  - Outputs all on the SP ring behind its inputs, pair-grouped with
    explicit (p, tile, d) 3-dim DRAM APs, singles for the last tiles.
  - Row sums: 11 tiles on ACT (activation+accum, ~2.3us each) and 5 on
    DVE via a Pool-assisted tree (Pool: t1=a+b FD1024 ~2.0us; DVE:
    t2=t1a+t1b FD512 ~0.33us + reduce FD512 ~0.6us) to unload DVE.
  - corr = U @ wt on PE: fp32 PSUM writeout is the TRN2 floor (bf16
    PSUM out is verifier-rejected), 64 x ~600ns. U^T matmuls grouped
    via quadrant-spread lhsT (ramp 1,1,2,3,3,3,3; rust allows partition
    bases {0,32,64} only): one matmul + one [*,128] psum->sbuf cast per
    group instead of 16 of each.
  - x += corr: 12 tiles direct DVE tensor_tensor from PSUM (1213ns per
    [128,1024]; PSUM caps DVE at 1x). 4 tiles use ACT copies
    PSUM->fp16 scratch (4 x ~600ns) + one all-16-bit add (2x: 1218ns
    per [128,2048]) on DVE (2 tiles) or Pool (2 late tiles; Pool bulk
    tt measured 4.0us/2048 and must not head-of-line block S builds,
    so only t>=12 and emitted in halves).
  - TensorTensor on ACT is illegal on TRN2 (NCC_IBIR606); Pool has no
    PSUM port and no scalar_tensor_tensor/partition_broadcast ISA;
    accum_out forces DVE ops to 1x. All verified on HW.
  - _sanitize_waits drops semaphore waits that are implied by engine
    FIFO order or transitively by the per-tile dependency chain
    (corr <- cast <- U^T <- S build <- row sum <- x load), using
    per-instruction category tags recorded at emission.
"""

import numpy as np
import ml_dtypes

import concourse.bass as bass
import concourse.tile as tile
from concourse import mybir
from concourse.bass_utils import run_bass_kernel_spmd

B, M, D, K = 512, 32, 2048, 3
COUPLING_STRENGTH = 0.1
SPATIAL_DECAY = 2.0

N_CORES = 8
B_SH = B // N_CORES          # 64 batches per core
ROWS = B_SH * M              # 2048 rows per core
P = 128                      # SBUF partitions
N_TILES = ROWS // P          # 16
F32 = mybir.dt.float32
F16 = mybir.dt.float16
BF16 = mybir.dt.bfloat16
ALU = mybir.AluOpType
ACTF = mybir.ActivationFunctionType

# ---- tunables -----------------------------------------------------------
GROUPS = [[0], [1], [2, 3], [4, 5, 6], [7, 8, 9], [10, 11, 12], [13, 14, 15]]
IN_ACT = {1, 3, 5, 7}                 # input tiles dispatched by ACT (rest SP)
SUM_DVE = {3, 7, 11, 13, 15}          # row sums via Pool-L1 + DVE tree
ADD_SCHEME = {5: "dve", 9: "dve", 12: "pool", 14: "pool"}  # rest: direct DVE
OUT_GROUPS = [[0, 1], [2, 3], [4, 5], [6, 7], [8, 9], [10, 11], [12, 13], [14], [15]]

_ENGINE_SEM = {
    "Activation": "Activation",
    "DVE": "DVE",
    "Pool": "Pool",
    "PE": "PE",
}


def _group_of(t):
    for gi, g in enumerate(GROUPS):
        if t in g:
            return gi, g.index(t)
    raise ValueError(t)


def _emit(tc, nc, x, wt, dk, out, tags):
    def tag(inst, cat):
        tags[inst.ins.name] = cat
        return inst

    with (
        tc.tile_pool(name="const", bufs=1) as const_pool,
        tc.tile_pool(name="xbuf", bufs=1) as xbuf_pool,
        tc.tile_pool(name="small", bufs=1) as small_pool,
        tc.tile_pool(name="psc", bufs=1, space="PSUM") as psc_pool,
    ):
        dk_sb = const_pool.tile([P, P], BF16)
        wtq_sb = const_pool.tile([67, D], BF16)

        x_all = xbuf_pool.tile([P, N_TILES * D], F16)
        t_all = small_pool.tile([P, 2 * N_TILES], F32)
        trash = small_pool.tile([P, D], F16)
        scratch = [
            small_pool.tile([P, D], F16, name=f"scr{i}") for i in range(2)
        ]
        t1buf = [
            small_pool.tile([P, 1024], F16, name=f"t1b{i}") for i in range(2)
        ]
        s_sp = [
            small_pool.tile([P, 32 * (len(g) - 1) + K], BF16, name=f"ssp{gi}")
            for gi, g in enumerate(GROUPS)
        ]
        u_sb = [
            small_pool.tile([32 * (len(g) - 1) + K, P], BF16, name=f"usb{gi}")
            for gi, g in enumerate(GROUPS)
        ]
        pscs = [
            psc_pool.tile([P, 1024], F32, name=f"pscs{i}") for i in range(4)
        ]

        # ---- input + const DMAs (no waits) ----
        n_chunks = [2] + [1] * (N_TILES - 1)

        def emit_in(t):
            eng = nc.scalar if t in IN_ACT else nc.sync
            ch = D // n_chunks[t]
            for h in range(n_chunks[t]):
                eng.dma_start(
                    out=x_all[:, t * D + h * ch : t * D + (h + 1) * ch],
                    in_=x[t * P : (t + 1) * P, h * ch : (h + 1) * ch],
                )

        emit_in(0)
        # consts right behind tile0 on the fast SP ring (44 KiB)
        nc.sync.dma_start(out=dk_sb, in_=dk)
        nc.sync.dma_start(out=wtq_sb[0:K, :], in_=wt)
        for t in range(1, N_TILES):
            emit_in(t)
        # quadrant replicas of wt at partition bases 32/64 (Pool SWDGE,
        # SBUF->SBUF, 12 KiB each)
        for j in (1, 2):
            nc.gpsimd.dma_start(
                out=wtq_sb[32 * j : 32 * j + K, :], in_=wtq_sb[0:K, :]
            )

        t1_i = 0

        def emit_front(t):
            """Row sum + S build (into the group spread tile) for tile t."""
            nonlocal t1_i
            x_sl = x_all[:, t * D : (t + 1) * D]
            ch = D // n_chunks[t]
            if t not in SUM_DVE:
                for h in range(n_chunks[t]):
                    tag(nc.scalar.activation(
                        out=trash[:, :ch],
                        in_=x_all[:, t * D + h * ch : t * D + (h + 1) * ch],
                        func=ACTF.Copy,
                        accum_out=t_all[:, 2 * t + h : 2 * t + h + 1],
                    ), "sum_act")
            else:
                t1 = t1buf[t1_i]
                t1_i ^= 1
                tag(nc.gpsimd.tensor_add(
                    t1[:], x_sl[:, 0:1024], x_sl[:, 1024:2048]
                ), "pool_l1")
                tag(nc.vector.tensor_add(
                    t1[:, 0:512], t1[:, 0:512], t1[:, 512:1024]
                ), "dve_t2")
                tag(nc.vector.tensor_reduce(
                    out=t_all[:, 2 * t : 2 * t + 1],
                    in_=t1[:, 0:512],
                    axis=mybir.AxisListType.X,
                    op=ALU.add,
                ), "dve_reduce")
            gi, j = _group_of(t)
            st = s_sp[gi][:, 32 * j : 32 * j + K]
            ta = t_all[:, 2 * t : 2 * t + 1]
            if n_chunks[t] == 1:
                tag(nc.gpsimd.tensor_copy(st[:, 1:2], ta), "pool_build")
            else:
                tag(nc.gpsimd.tensor_add(
                    st[:, 1:2], ta, t_all[:, 2 * t + 1 : 2 * t + 2]
                ), "pool_build")
            tag(nc.gpsimd.tensor_sub(
                st[:, 0:1], st[:, 1:2], x_sl[:, D - 1 : D]
            ), "pool_build")
            tag(nc.gpsimd.tensor_sub(
                st[:, 2:3], st[:, 1:2], x_sl[:, 0:1]
            ), "pool_build")

        def emit_group_u(gi):
            g = GROUPS[gi]
            rows = 32 * (len(g) - 1) + K
            ut_ps = pscs[(2 * g[0]) % 4][0:rows, 0:P]
            nc.tensor.matmul(
                ut_ps, lhsT=s_sp[gi][:], rhs=dk_sb, start=True, stop=True
            )
            tag(nc.vector.tensor_copy(u_sb[gi][:], ut_ps), "dve_cast")

        scr_state = [0]

        def emit_add(t):
            x_sl = x_all[:, t * D : (t + 1) * D]
            mode = ADD_SCHEME.get(t)
            if mode is None:
                for c in range(2):
                    ps = pscs[(2 * t + c) % 4]
                    tag(nc.vector.tensor_add(
                        x_sl[:, c * 1024 : (c + 1) * 1024],
                        x_sl[:, c * 1024 : (c + 1) * 1024],
                        ps,
                    ), "dve_psadd")
            else:
                scr = scratch[scr_state[0]]
                scr_state[0] ^= 1
                for c in range(2):
                    ps = pscs[(2 * t + c) % 4]
                    for n in range(2):
                        tag(nc.scalar.activation(
                            out=scr[:, c * 1024 + n * 512 : c * 1024 + (n + 1) * 512],
                            in_=ps[:, n * 512 : (n + 1) * 512],
                            func=ACTF.Copy,
                        ), "act_copy")
                if mode == "dve":
                    tag(nc.vector.tensor_tensor(
                        out=x_sl, in0=x_sl, in1=scr[:], op=ALU.add
                    ), "dve_scradd")
                else:
                    for c in range(2):
                        tag(nc.gpsimd.tensor_tensor(
                            out=x_sl[:, c * 1024 : (c + 1) * 1024],
                            in0=x_sl[:, c * 1024 : (c + 1) * 1024],
                            in1=scr[:, c * 1024 : (c + 1) * 1024],
                            op=ALU.add,
                        ), "pool_scradd")

        def emit_corr(t):
            gi, j = _group_of(t)
            u_sl = u_sb[gi][32 * j : 32 * j + K, :]
            wt_sl = wtq_sb[32 * j : 32 * j + K, :]
            for c in range(2):
                ps = pscs[(2 * t + c) % 4]
                for n in range(2):
                    col0 = c * 1024 + n * 512
                    nc.tensor.matmul(
                        ps[:, n * 512 : (n + 1) * 512],
                        lhsT=u_sl,
                        rhs=wt_sl[:, col0 : col0 + 512],
                        start=True,
                        stop=True,
                    )

        emitted_u = set()

        def maybe_emit_u(upto_tile):
            for gi, g in enumerate(GROUPS):
                if gi not in emitted_u and g[-1] <= upto_tile:
                    emit_group_u(gi)
                    emitted_u.add(gi)

        emit_front(0)
        emit_front(1)
        maybe_emit_u(1)
        for t in range(N_TILES):
            if t + 2 < N_TILES:
                emit_front(t + 2)
                maybe_emit_u(t + 2)
            emit_corr(t)
            emit_add(t)
        for grp in OUT_GROUPS:
            t0 = grp[0]
            nt = len(grp)
            if nt == 1:
                dst = out[t0 * P : (t0 + 1) * P, :]
            else:
                dst = bass.AP(
                    out,
                    t0 * P * D,
                    [[D, P], [P * D, nt], [1, D]],
                )
            nc.sync.dma_start(
                out=dst,
                in_=x_all[:, t0 * D : (t0 + nt) * D],
            )


_NC_CACHE = None


def _build_nc():
    global _NC_CACHE
    if _NC_CACHE is not None:
        return _NC_CACHE
    nc = bass.Bass()
    x = nc.declare_dram_parameter("x", [ROWS, D], F16, isOutput=False)
    wt = nc.declare_dram_parameter("wt", [K, D], BF16, isOutput=False)
    dk = nc.declare_dram_parameter("dk", [P, P], BF16, isOutput=False)
    out = nc.declare_dram_parameter("out", [ROWS, D], F16, isOutput=True)
    tags: dict[str, str] = {}
    with tile.TileContext(nc) as tc:
        _emit(tc, nc, x[:], wt[:], dk[:], out, tags)
    _sanitize_waits(nc, tags)
    _NC_CACHE = nc
    return nc


# category -> semaphore-name prefixes whose waits must be KEPT (others are
# implied transitively; see docstring). None = keep all cross-engine waits.
_KEEP = {
    "sum_act": ("DMA",),
    "pool_l1": ("DMA",),
    "dve_t2": ("Pool",),
    "dve_reduce": ("Pool", "DVE"),
    "pool_build": ("Activation", "DVE"),
    "dve_cast": ("PE",),
    "dve_psadd": ("PE",),
    "act_copy": ("PE", "DVE", "Pool"),
    "dve_scradd": ("Activation",),
    "pool_scradd": ("Activation",),
}


def _sanitize_waits(nc, tags):
    """One semaphore wait per instruction (HW has a single wait slot).

    1. Drop engine self-waits (engine FIFO implies them; no op here reads
       a location a prior op on the same engine wrote in a hazardous way).
    2. Drop cross-engine waits implied transitively through the per-tile
       chain, per the _KEEP table keyed by emission-time category tags.
    3. Split any remaining multi-wait instruction into standalone
       InstEventSemaphore ops on the same queue.
    """
    from concourse import mybir as _mb

    skip = ("InstEventSemaphore", "InstAllEngineBarrier")
    for f in nc.m.functions:
        for bb in f.blocks:
            idx = 0
            insts = bb.instructions
            while idx < len(insts):
                inst = insts[idx]
                si = inst.sync_info
                cls = type(inst).__name__
                eng = str(inst.engine)
                if si is not None and si.on_wait and cls not in skip:
                    waits = list(si.on_wait)
                    own = _ENGINE_SEM.get(eng.split(".")[-1])
                    if own is not None:
                        waits = [
                            w for w in waits if not w.ant_name.startswith(own)
                        ] or waits[-1:]
                    keep = _KEEP.get(tags.get(inst.name))
                    if keep is not None:
                        kept = [
                            w
                            for w in waits
                            if any(w.ant_name.startswith(k) for k in keep)
                        ]
                        if kept:
                            waits = kept
                    if cls == "InstMatmult":
                        kept = [
                            w for w in waits if not w.ant_name.startswith("PE")
                        ]
                        if kept:
                            waits = kept
                    if len(waits) != len(si.on_wait):
                        si = _mb.SyncInfo(
                            on_wait=waits, on_update=list(si.on_update)
                        )
                        inst.sync_info = si
                if (
                    si is None
                    or not si.on_wait
                    or len(si.on_wait) < 2
                    or cls in skip
                ):
                    idx += 1
                    continue
                waits = list(si.on_wait)
                n_new = 0
                for w in waits[:-1]:
                    ev = _mb.InstEventSemaphore(
                        name=nc.get_next_instruction_name(), ins=[], outs=[]
                    )
                    ev.engine = inst.engine
                    ev.sync_info = _mb.SyncInfo(on_wait=[w], on_update=[])
                    nc.register_instruction(ev)
                    insts.insert(idx + n_new, ev)
                    n_new += 1
                inst.sync_info = _mb.SyncInfo(
                    on_wait=[waits[-1]], on_update=list(si.on_update)
                )
                idx += n_new + 1


def _host_constants(w: np.ndarray):
    wt = np.ascontiguousarray(
        w.T.astype(np.float32).astype(ml_dtypes.bfloat16)
    )  # [3, D] raw
    idx = np.arange(M)
    dec = np.exp(-np.abs(idx[:, None] - idx[None, :]) / SPATIAL_DECAY)
    dec = (dec * (1.0 - np.eye(M))) * (COUPLING_STRENGTH / D)
    dk = np.ascontiguousarray(
        np.kron(np.eye(P // M, dtype=np.float64), dec).astype(np.float32)
        .astype(ml_dtypes.bfloat16)
    )
    return wt, dk


def _in_maps(x: np.ndarray, w: np.ndarray):
    wt, dk = _host_constants(np.asarray(w, dtype=np.float32))
    x16 = np.asarray(x, dtype=np.float16).reshape(N_CORES, ROWS, D)
    return [
        {"x": np.ascontiguousarray(x16[i]), "wt": wt, "dk": dk}
        for i in range(N_CORES)
    ]


def kernel(x: np.ndarray, w: np.ndarray, _results_out: list | None = None) -> np.ndarray:
    nc = _build_nc()
    res = run_bass_kernel_spmd(nc, _in_maps(x, w), core_ids=list(range(N_CORES)))
    if _results_out is not None:
        _results_out.append(res)
    out = np.concatenate(
        [np.asarray(res.results[i]["out"]).reshape(B_SH, M, D) for i in range(N_CORES)],
        axis=0,
    )
    return out.astype(np.float32)


# revision 9
# speedup vs baseline: 1.2427x; 1.0524x over previous
"""Trainium2 Bass kernel for nn_EphapticCoupling_51857435132573 (v2).

Math: for x[B,M,D], w[D,K=3] the reference collapses to a rank-3 update
    out[b,m,d] = x[b,m,d] + sum_k U[b,m,k] * wt[k,d]
    U  = decay @ S,  S = [T - x[..,-1], T, T - x[..,0]],  T = x.sum(-1)
    (0.1/D scale folded into dk)

I/O precision: gate is rel_err < 2e-2; host feeds x fp16 and reads the
output fp16 (8 MiB in + 8 MiB out per core).

Architecture (every cost HW-measured this session):
  - Inputs split across BOTH HWDGE rings (12 tiles qSP + 4 tiles qAct;
    single ring sustains ~321 GB/s vs ~358 HBM/NC cap). In-dispatches
    carry no waits so ACT's 4 never delay its compute by much.
  - dk + wt consts ride the SP ring right after tile0 (44 KiB; the
    gpsimd software queue measured only ~10 B/ns and starved corr).
    wt is [3,2048]; Pool replicates it to partition bases 32/64 via
    SBUF->SBUF DMAs for the quadrant corr matmuls.
  - Outputs all on the SP ring behind its inputs, pair-grouped with
    explicit (p, tile, d) 3-dim DRAM APs, singles for the last tiles.
  - Row sums: 11 tiles on ACT (activation+accum, ~2.3us each) and 5 on
    DVE via a Pool-assisted tree (Pool: t1=a+b FD1024 ~2.0us; DVE:
    t2=t1a+t1b FD512 ~0.33us + reduce FD512 ~0.6us) to unload DVE.
  - corr = U @ wt on PE: fp32 PSUM writeout is the TRN2 floor (bf16
    PSUM out is verifier-rejected), 64 x ~600ns. U^T matmuls grouped
    via quadrant-spread lhsT (ramp 1,1,2,3,3,3,3; rust allows partition
    bases {0,32,64} only): one matmul + one [*,128] psum->sbuf cast per
    group instead of 16 of each.
  - x += corr: 12 tiles direct DVE tensor_tensor from PSUM (1213ns per
    [128,1024]; PSUM caps DVE at 1x). 4 tiles use ACT copies
    PSUM->fp16 scratch (4 x ~600ns) + one all-16-bit add (2x: 1218ns
    per [128,2048]) on DVE (2 tiles) or Pool (2 late tiles; Pool bulk
    tt measured 4.0us/2048 and must not head-of-line block S builds,
    so only t>=12 and emitted in halves).
  - TensorTensor on ACT is illegal on TRN2 (NCC_IBIR606); Pool has no
    PSUM port and no scalar_tensor_tensor/partition_broadcast ISA;
    accum_out forces DVE ops to 1x. All verified on HW.
  - _sanitize_waits drops semaphore waits that are implied by engine
    FIFO order or transitively by the per-tile dependency chain
    (corr <- cast <- U^T <- S build <- row sum <- x load), using
    per-instruction category tags recorded at emission.
"""

import numpy as np
import ml_dtypes

import concourse.bass as bass
import concourse.tile as tile
from concourse import mybir
from concourse.bass_utils import run_bass_kernel_spmd

B, M, D, K = 512, 32, 2048, 3
COUPLING_STRENGTH = 0.1
SPATIAL_DECAY = 2.0

N_CORES = 8
B_SH = B // N_CORES          # 64 batches per core
ROWS = B_SH * M              # 2048 rows per core
P = 128                      # SBUF partitions
N_TILES = ROWS // P          # 16
F32 = mybir.dt.float32
F16 = mybir.dt.float16
BF16 = mybir.dt.bfloat16
ALU = mybir.AluOpType
ACTF = mybir.ActivationFunctionType

# ---- tunables -----------------------------------------------------------
GROUPS = [[0], [1], [2, 3], [4, 5, 6], [7, 8, 9], [10, 11, 12], [13, 14, 15]]
IN_ACT = [1, 3, 5, 7]                 # input tiles dispatched by ACT (rest SP)
SUM_DVE = {3, 7, 11, 15}              # row sums via Pool-L1 + DVE tree
ADD_SCHEME = {12: "pool", 14: "pool"}  # rest: direct DVE
OUT_SP = [[0, 1], [2, 3], [4, 5], [6, 7], [8, 9], [10, 11]]
OUT_ACT = [[12, 13], [14], [15]]      # dispatched at the end of ACT's stream

_ENGINE_SEM = {
    "Activation": "Activation",
    "DVE": "DVE",
    "Pool": "Pool",
    "PE": "PE",
}


def _group_of(t):
    for gi, g in enumerate(GROUPS):
        if t in g:
            return gi, g.index(t)
    raise ValueError(t)


def _emit(tc, nc, x, wt, dk, out, tags):
    def tag(inst, cat):
        tags[inst.ins.name] = cat
        return inst

    with (
        tc.tile_pool(name="const", bufs=1) as const_pool,
        tc.tile_pool(name="xbuf", bufs=1) as xbuf_pool,
        tc.tile_pool(name="small", bufs=1) as small_pool,
        tc.tile_pool(name="psc", bufs=1, space="PSUM") as psc_pool,
    ):
        dk_sb = const_pool.tile([P, P], BF16)
        wtq_sb = const_pool.tile([67, D], BF16)

        x_all = xbuf_pool.tile([P, N_TILES * D], F16)
        t_all = small_pool.tile([P, 2 * N_TILES], F32)
        trash = small_pool.tile([P, D], F16)
        scratch = [
            small_pool.tile([P, D], F16, name=f"scr{i}") for i in range(2)
        ]
        t1buf = [
            small_pool.tile([P, 1024], F16, name=f"t1b{i}") for i in range(2)
        ]
        s_sp = [
            small_pool.tile([P, 32 * (len(g) - 1) + K], BF16, name=f"ssp{gi}")
            for gi, g in enumerate(GROUPS)
        ]
        u_sb = [
            small_pool.tile([32 * (len(g) - 1) + K, P], BF16, name=f"usb{gi}")
            for gi, g in enumerate(GROUPS)
        ]
        pscs = [
            psc_pool.tile([P, 1024], F32, name=f"pscs{i}") for i in range(4)
        ]

        # ---- input + const DMAs (no waits) ----
        n_chunks = [2] + [1] * (N_TILES - 1)

        def emit_in(t):
            eng = nc.scalar if t in IN_ACT else nc.sync
            ch = D // n_chunks[t]
            for h in range(n_chunks[t]):
                eng.dma_start(
                    out=x_all[:, t * D + h * ch : t * D + (h + 1) * ch],
                    in_=x[t * P : (t + 1) * P, h * ch : (h + 1) * ch],
                )

        emit_in(0)
        emit_in(IN_ACT[0])
        # consts right behind tile0 on the fast SP ring (44 KiB)
        nc.sync.dma_start(out=dk_sb, in_=dk)
        nc.sync.dma_start(out=wtq_sb[0:K, :], in_=wt)
        for t in range(1, N_TILES):
            if t not in IN_ACT:
                emit_in(t)
        # quadrant replicas of wt at partition bases 32/64 (Pool SWDGE,
        # SBUF->SBUF, 12 KiB each)
        for j in (1, 2):
            nc.gpsimd.dma_start(
                out=wtq_sb[32 * j : 32 * j + K, :], in_=wtq_sb[0:K, :]
            )
        in_act_left = list(IN_ACT[1:])

        t1_i = 0

        def emit_front(t):
            """Row sum + S build (into the group spread tile) for tile t."""
            nonlocal t1_i
            x_sl = x_all[:, t * D : (t + 1) * D]
            ch = D // n_chunks[t]
            if t not in SUM_DVE:
                for h in range(n_chunks[t]):
                    tag(nc.scalar.activation(
                        out=trash[:, :ch],
                        in_=x_all[:, t * D + h * ch : t * D + (h + 1) * ch],
                        func=ACTF.Copy,
                        accum_out=t_all[:, 2 * t + h : 2 * t + h + 1],
                    ), "sum_act")
            else:
                t1 = t1buf[t1_i]
                t1_i ^= 1
                tag(nc.gpsimd.tensor_add(
                    t1[:], x_sl[:, 0:1024], x_sl[:, 1024:2048]
                ), "pool_l1")
                tag(nc.vector.tensor_add(
                    t1[:, 0:512], t1[:, 0:512], t1[:, 512:1024]
                ), "dve_t2")
                tag(nc.vector.tensor_reduce(
                    out=t_all[:, 2 * t : 2 * t + 1],
                    in_=t1[:, 0:512],
                    axis=mybir.AxisListType.X,
                    op=ALU.add,
                ), "dve_reduce")
            gi, j = _group_of(t)
            st = s_sp[gi][:, 32 * j : 32 * j + K]
            ta = t_all[:, 2 * t : 2 * t + 1]
            with tc.high_priority(offset=40):
                if n_chunks[t] == 1:
                    tag(nc.gpsimd.tensor_copy(st[:, 1:2], ta), "pool_build")
                else:
                    tag(nc.gpsimd.tensor_add(
                        st[:, 1:2], ta, t_all[:, 2 * t + 1 : 2 * t + 2]
                    ), "pool_build")
                tag(nc.gpsimd.tensor_sub(
                    st[:, 0:1], st[:, 1:2], x_sl[:, D - 1 : D]
                ), "pool_build")
                tag(nc.gpsimd.tensor_sub(
                    st[:, 2:3], st[:, 1:2], x_sl[:, 0:1]
                ), "pool_build")

        def emit_group_u(gi):
            g = GROUPS[gi]
            rows = 32 * (len(g) - 1) + K
            ut_ps = pscs[(2 * g[0]) % 4][0:rows, 0:P]
            with tc.high_priority(offset=40):
                nc.tensor.matmul(
                    ut_ps, lhsT=s_sp[gi][:], rhs=dk_sb, start=True, stop=True
                )
                tag(nc.scalar.activation(
                    out=u_sb[gi][:], in_=ut_ps, func=ACTF.Copy
                ), "act_cast")

        scr_state = [0]

        def emit_add(t):
            x_sl = x_all[:, t * D : (t + 1) * D]
            mode = ADD_SCHEME.get(t)
            if mode is None:
                for c in range(2):
                    ps = pscs[(2 * t + c) % 4]
                    tag(nc.vector.tensor_add(
                        x_sl[:, c * 1024 : (c + 1) * 1024],
                        x_sl[:, c * 1024 : (c + 1) * 1024],
                        ps,
                    ), "dve_psadd")
            else:
                scr = scratch[scr_state[0]]
                scr_state[0] ^= 1
                for c in range(2):
                    ps = pscs[(2 * t + c) % 4]
                    for n in range(2):
                        tag(nc.scalar.activation(
                            out=scr[:, c * 1024 + n * 512 : c * 1024 + (n + 1) * 512],
                            in_=ps[:, n * 512 : (n + 1) * 512],
                            func=ACTF.Copy,
                        ), "act_copy")
                if mode == "dve":
                    tag(nc.vector.tensor_tensor(
                        out=x_sl, in0=x_sl, in1=scr[:], op=ALU.add
                    ), "dve_scradd")
                else:
                    for c in range(2):
                        tag(nc.gpsimd.tensor_tensor(
                            out=x_sl[:, c * 1024 : (c + 1) * 1024],
                            in0=x_sl[:, c * 1024 : (c + 1) * 1024],
                            in1=scr[:, c * 1024 : (c + 1) * 1024],
                            op=ALU.add,
                        ), "pool_scradd")

        def emit_corr(t):
            gi, j = _group_of(t)
            u_sl = u_sb[gi][32 * j : 32 * j + K, :]
            wt_sl = wtq_sb[32 * j : 32 * j + K, :]
            for c in range(2):
                ps = pscs[(2 * t + c) % 4]
                for n in range(2):
                    col0 = c * 1024 + n * 512
                    nc.tensor.matmul(
                        ps[:, n * 512 : (n + 1) * 512],
                        lhsT=u_sl,
                        rhs=wt_sl[:, col0 : col0 + 512],
                        start=True,
                        stop=True,
                    )

        emitted_u = set()

        def maybe_emit_u(upto_tile):
            for gi, g in enumerate(GROUPS):
                if gi not in emitted_u and g[-1] <= upto_tile:
                    emit_group_u(gi)
                    emitted_u.add(gi)

        def out_ap(grp):
            t0 = grp[0]
            nt = len(grp)
            if nt == 1:
                return out[t0 * P : (t0 + 1) * P, :], t0, nt
            return (
                bass.AP(out, t0 * P * D, [[D, P], [P * D, nt], [1, D]]),
                t0,
                nt,
            )

        emit_front(0)
        maybe_emit_u(0)
        emit_front(1)
        maybe_emit_u(1)
        for t in range(N_TILES):
            if in_act_left:
                emit_in(in_act_left.pop(0))
            if t + 2 < N_TILES:
                emit_front(t + 2)
                maybe_emit_u(t + 2)
            emit_corr(t)
            emit_add(t)
        for grp in OUT_SP:
            dst, t0, nt = out_ap(grp)
            nc.sync.dma_start(out=dst, in_=x_all[:, t0 * D : (t0 + nt) * D])
        for grp in OUT_ACT:
            dst, t0, nt = out_ap(grp)
            nc.scalar.dma_start(out=dst, in_=x_all[:, t0 * D : (t0 + nt) * D])


_NC_CACHE = None


def _build_nc():
    global _NC_CACHE
    if _NC_CACHE is not None:
        return _NC_CACHE
    nc = bass.Bass()
    x = nc.declare_dram_parameter("x", [ROWS, D], F16, isOutput=False)
    wt = nc.declare_dram_parameter("wt", [K, D], BF16, isOutput=False)
    dk = nc.declare_dram_parameter("dk", [P, P], BF16, isOutput=False)
    out = nc.declare_dram_parameter("out", [ROWS, D], F16, isOutput=True)
    tags: dict[str, str] = {}
    with tile.TileContext(nc) as tc:
        _emit(tc, nc, x[:], wt[:], dk[:], out, tags)
    _sanitize_waits(nc, tags)
    _NC_CACHE = nc
    return nc


# category -> semaphore-name prefixes whose waits must be KEPT (others are
# implied transitively; see docstring). None = keep all cross-engine waits.
_KEEP = {
    "sum_act": ("DMA",),
    "pool_l1": ("DMA",),
    "dve_t2": ("Pool",),
    "dve_reduce": ("Pool", "DVE"),
    "pool_build": ("Activation", "DVE"),
    "act_cast": ("PE",),
    "dve_psadd": ("PE",),
    "act_copy": ("PE", "DVE", "Pool"),
    "dve_scradd": ("Activation",),
    "pool_scradd": ("Activation",),
}


def _sanitize_waits(nc, tags):
    """One semaphore wait per instruction (HW has a single wait slot).

    1. Drop engine self-waits (engine FIFO implies them; no op here reads
       a location a prior op on the same engine wrote in a hazardous way).
    2. Drop cross-engine waits implied transitively through the per-tile
       chain, per the _KEEP table keyed by emission-time category tags.
    3. Split any remaining multi-wait instruction into standalone
       InstEventSemaphore ops on the same queue.
    """
    from concourse import mybir as _mb

    skip = ("InstEventSemaphore", "InstAllEngineBarrier")
    for f in nc.m.functions:
        for bb in f.blocks:
            idx = 0
            insts = bb.instructions
            while idx < len(insts):
                inst = insts[idx]
                si = inst.sync_info
                cls = type(inst).__name__
                eng = str(inst.engine)
                if si is not None and si.on_wait and cls not in skip:
                    waits = list(si.on_wait)
                    own = _ENGINE_SEM.get(eng.split(".")[-1])
                    if own is not None:
                        waits = [
                            w for w in waits if not w.ant_name.startswith(own)
                        ] or waits[-1:]
                    keep = _KEEP.get(tags.get(inst.name))
                    if keep is not None:
                        kept = [
                            w
                            for w in waits
                            if any(w.ant_name.startswith(k) for k in keep)
                        ]
                        if kept:
                            waits = kept
                    if cls == "InstMatmult":
                        kept = [
                            w for w in waits if not w.ant_name.startswith("PE")
                        ]
                        if kept:
                            waits = kept
                    if len(waits) != len(si.on_wait):
                        si = _mb.SyncInfo(
                            on_wait=waits, on_update=list(si.on_update)
                        )
                        inst.sync_info = si
                if (
                    si is None
                    or not si.on_wait
                    or len(si.on_wait) < 2
                    or cls in skip
                ):
                    idx += 1
                    continue
                waits = list(si.on_wait)
                n_new = 0
                for w in waits[:-1]:
                    ev = _mb.InstEventSemaphore(
                        name=nc.get_next_instruction_name(), ins=[], outs=[]
                    )
                    ev.engine = inst.engine
                    ev.sync_info = _mb.SyncInfo(on_wait=[w], on_update=[])
                    nc.register_instruction(ev)
                    insts.insert(idx + n_new, ev)
                    n_new += 1
                inst.sync_info = _mb.SyncInfo(
                    on_wait=[waits[-1]], on_update=list(si.on_update)
                )
                idx += n_new + 1


def _host_constants(w: np.ndarray):
    wt = np.ascontiguousarray(
        w.T.astype(np.float32).astype(ml_dtypes.bfloat16)
    )  # [3, D] raw
    idx = np.arange(M)
    dec = np.exp(-np.abs(idx[:, None] - idx[None, :]) / SPATIAL_DECAY)
    dec = (dec * (1.0 - np.eye(M))) * (COUPLING_STRENGTH / D)
    dk = np.ascontiguousarray(
        np.kron(np.eye(P // M, dtype=np.float64), dec).astype(np.float32)
        .astype(ml_dtypes.bfloat16)
    )
    return wt, dk


def _in_maps(x: np.ndarray, w: np.ndarray):
    wt, dk = _host_constants(np.asarray(w, dtype=np.float32))
    x16 = np.asarray(x, dtype=np.float16).reshape(N_CORES, ROWS, D)
    return [
        {"x": np.ascontiguousarray(x16[i]), "wt": wt, "dk": dk}
        for i in range(N_CORES)
    ]


def kernel(x: np.ndarray, w: np.ndarray, _results_out: list | None = None) -> np.ndarray:
    nc = _build_nc()
    res = run_bass_kernel_spmd(nc, _in_maps(x, w), core_ids=list(range(N_CORES)))
    if _results_out is not None:
        _results_out.append(res)
    out = np.concatenate(
        [np.asarray(res.results[i]["out"]).reshape(B_SH, M, D) for i in range(N_CORES)],
        axis=0,
    )
    return out.astype(np.float32)
